# revision 9
# baseline (speedup 1.0000x reference)
"""Trainium2 Bass kernel for the dual-branch CustomLSTMCell.

Math (reference):
    hx_l = [h_light | y]  [B, H+I]     hx_t = [h_temp | y]
    z_br = hx_br @ W_br.T + b_br       (W_br = vstack(w_f,w_i,w_c,w_o) [4H, H+I])
    f,i,ch,o = sigmoid/sigmoid/tanh/sigmoid splits of z_br
    c_new = (f1 + f2) * c_light + i1*ch1 + i2*ch2      (c_temp is unused)
    h_new = (o1 + o2) * tanh(c_new)

Strategy: 2D shard over 8 NeuronCores — 4-way batch x 2-way hidden, no
collectives (each core owns a disjoint (batch, hidden) tile of h/c, and
gate row j only needs the hx rows the core already has). Per core:
batch 1024 (2 moving chunks of 512), hidden 512 per gate (4 row-tiles).

The GEMMs run in fp16 (1 PE cycle/row; fp32 would be 4x slower, and
fp8-DoubleRow's 2x FLOP rate cannot pay for the 3x FLOPs a split-precision
scheme needs to stay under the accuracy gate — measured 271us vs this
design's PE floor of 164us). Per core we compute z.T tiles: psum[zcol 128,
batch 512] = Wtile[K=128, M=128].T @ hxT[K=128, N=512], accumulated over
K=1536 (12 k-tiles). vs pure data-parallel, the 2D shard halves the weight
stream (12.6 MB/core fp16), so DMA (~21 MB/core total) sits well under the
PE time and stays fully overlapped.

Gate bias + sigmoid/tanh run on the Scalar engine straight out of PSUM
(bias is per-partition in this transposed layout), the LSTM cell
elementwise runs on the Vector engine in fp32, results DMA out transposed,
and the host un-transposes. All transposes/casts happen host-side so every
device DMA is a contiguous 2D partition-major stream.

Schedule notes: weight DMAs are issued with a software prefetch distance
of PREFETCH units; the first weight unit is split into k-chunks so the
first matmul only waits on a 33KB slice. A short PE pre-warm (dummy
matmuls on a zeroed tile) burns the DVFS p-state ramp while the first
operands are in flight. The per-branch gate order is (i, c, f, o) so the
output gate of the temp branch — the only input of the final h_new chain —
finishes last and the tail after the final matmul is short.
"""

import os
import sys

for _p in ("/opt/trn_rl_repo",):
    if os.path.isdir(_p) and _p not in sys.path:
        sys.path.append(_p)

import numpy as np

import concourse.bass as bass
import concourse.mybir as mybir
import concourse.tile as tile
from concourse import bacc
from concourse.bass_utils import run_bass_kernel_spmd

B, I, H = 4096, 512, 1024
N_CORES = 8
SB, SH = 4, 2              # batch x hidden core grid
BS = B // SB               # 1024 batch rows per core
CH = 2                     # batch chunks per core
CB = BS // CH              # 512 moving rows per matmul
HS = H // SH               # 512 hidden outputs per gate per core
RT = HS // 128             # 4 hidden row-tiles
K = H + I                  # 1536 contraction
KT = K // 128              # 12 k-tiles
N_U = RT * 2 * 4           # 32 weight units: (r, branch, gate)
GATE_ORDER = (1, 2, 0, 3)  # i, c, f, o
PREFETCH = 5               # weight units in flight ahead of use
N_WARM = 12                # PE pre-warm matmuls (128-cycle each)

_F32 = mybir.dt.float32
_F16 = mybir.dt.float16
AF = mybir.ActivationFunctionType
F16 = np.float16


def _build_nc():
    nc = bacc.Bacc("TRN2", target_bir_lowering=False, debug=False,
                   enable_asserts=False)

    wp = nc.dram_tensor("wp", [N_U, 128, KT * 128], _F16,
                        kind="ExternalInput")
    ap = nc.dram_tensor("ap", [2, CH, 128, KT * CB], _F16,
                        kind="ExternalInput")
    bp = nc.dram_tensor("bp", [128, N_U], _F32, kind="ExternalInput")
    ct = nc.dram_tensor("ct", [RT, CH, 128, CB], _F32, kind="ExternalInput")
    h_out = nc.dram_tensor("h_out", [RT, CH, 128, CB], _F32,
                           kind="ExternalOutput")
    c_out = nc.dram_tensor("c_out", [RT, CH, 128, CB], _F32,
                           kind="ExternalOutput")

    with tile.TileContext(nc) as tc:
        with (
            tc.tile_pool(name="const", bufs=1) as cpool,
            tc.tile_pool(name="w", bufs=PREFETCH + 3) as wpool,
            tc.tile_pool(name="gates", bufs=18) as gpool,
            tc.tile_pool(name="cin", bufs=4) as cin_pool,
            tc.tile_pool(name="ew", bufs=4) as epool,
            tc.tile_pool(name="out", bufs=4) as opool,
            tc.tile_pool(name="psum", bufs=8, space="PSUM") as pspool,
        ):
            wt_tiles = {}

            def issue_wt(seq, eng=None):
                t = wpool.tile([128, KT * 128], _F16, tag="w")
                (eng or nc.sync).dma_start(out=t[:], in_=wp[seq])
                wt_tiles[seq] = t

            # PE pre-warm: dummy matmuls on a zeroed tile burn the DVFS
            # p-state ramp while the first operands are in flight.
            warm = cpool.tile([128, 128], _F16, tag="warm")
            nc.gpsimd.memset(warm[:], 0.0)
            wpsum = pspool.tile([128, 512], _F32, tag="pt")
            for _ in range(N_WARM):
                nc.tensor.matmul(wpsum[:, 0:128], warm[:], warm[:],
                                 start=True, stop=True)

            # startup. Queue discipline: the sync queue carries ONLY the
            # weight stream (so weight unit u never waits behind bulk
            # activation bytes); gpsimd and scalar queues split the
            # activation tiles, bias, c_light, and the output writes.
            wt0 = wpool.tile([128, KT * 128], _F16, tag="w")
            for k in range(KT):
                nc.sync.dma_start(out=wt0[:, bass.ts(k, 128)],
                                  in_=wp[0][:, bass.ts(k, 128)])
            wt_tiles[0] = wt0
            issue_wt(1)
            a_sb = {}
            for br in range(2):
                for ch in range(CH):
                    a_tile = cpool.tile([128, KT * CB], _F16,
                                        tag=f"a{br}{ch}")
                    a_sb[(br, ch)] = a_tile
            bias_sb = cpool.tile([128, N_U], _F32, tag="bias")
            nc.scalar.dma_start(out=bias_sb[:], in_=bp[:])
            for k in range(KT):
                nc.gpsimd.dma_start(out=a_sb[(0, 0)][:, bass.ts(k, CB)],
                                    in_=ap[0, 0][:, bass.ts(k, CB)])
                nc.scalar.dma_start(out=a_sb[(0, 1)][:, bass.ts(k, CB)],
                                    in_=ap[0, 1][:, bass.ts(k, CB)])
            issue_wt(2)
            issue_wt(3)
            nc.gpsimd.dma_start(out=a_sb[(1, 0)][:], in_=ap[1, 0])
            nc.scalar.dma_start(out=a_sb[(1, 1)][:], in_=ap[1, 1])
            issue_wt(4)

            seq = 0  # weight-unit index (matches host pack order)
            for r in range(RT):
                ct_t = []
                for ch in range(CH):
                    t = cin_pool.tile([128, CB], _F32, tag="ct")
                    nc.scalar.dma_start(out=t[:], in_=ct[r, ch])
                    ct_t.append(t)

                gates = {}
                for br in range(2):
                    for g in GATE_ORDER:
                        if seq + PREFETCH < N_U:
                            issue_wt(seq + PREFETCH)
                        wt = wt_tiles.pop(seq)
                        func = AF.Tanh if g == 2 else AF.Sigmoid
                        for ch in range(CH):
                            a_t = a_sb[(br, ch)]
                            pt = pspool.tile([128, CB], _F32, tag="pt")
                            for k in range(KT):
                                nc.tensor.matmul(
                                    pt[:],
                                    wt[:, bass.ts(k, 128)],
                                    a_t[:, bass.ts(k, CB)],
                                    start=(k == 0),
                                    stop=(k == KT - 1),
                                )
                            gt = gpool.tile([128, CB], _F32, tag="gate")
                            nc.scalar.activation(gt[:], pt[:], func,
                                                 bias=bias_sb[:, seq:seq + 1],
                                                 scale=1.0)
                            gates[(br, g, ch)] = gt
                        seq += 1

                for ch in range(CH):
                    f1, i1, ch1, o1 = (gates[(0, g, ch)] for g in range(4))
                    f2, i2, ch2, o2 = (gates[(1, g, ch)] for g in range(4))

                    t_a = epool.tile([128, CB], _F32, tag="ta")
                    t_b = epool.tile([128, CB], _F32, tag="tb")
                    t_c = epool.tile([128, CB], _F32, tag="tc")
                    c_new = opool.tile([128, CB], _F32, tag="cn")
                    nc.vector.tensor_mul(t_b[:], i1[:], ch1[:])
                    nc.vector.tensor_mul(t_c[:], i2[:], ch2[:])
                    nc.vector.tensor_add(t_b[:], t_b[:], t_c[:])
                    nc.vector.tensor_add(t_a[:], f1[:], f2[:])
                    nc.vector.tensor_mul(t_a[:], t_a[:], ct_t[ch][:])
                    nc.vector.tensor_add(c_new[:], t_a[:], t_b[:])

                    th = epool.tile([128, CB], _F32, tag="th")
                    nc.scalar.activation(th[:], c_new[:], AF.Tanh)
                    h_new = opool.tile([128, CB], _F32, tag="hn")
                    nc.vector.tensor_add(t_a[:], o1[:], o2[:])
                    nc.vector.tensor_mul(h_new[:], t_a[:], th[:])

                    nc.gpsimd.dma_start(out=c_out[r, ch], in_=c_new[:])
                    nc.scalar.dma_start(out=h_out[r, ch], in_=h_new[:])

    nc.compile()
    return nc


_NC_CACHE = None


def _get_nc():
    global _NC_CACHE
    if _NC_CACHE is None:
        _NC_CACHE = _build_nc()
    return _NC_CACHE


def _pack_weights(inputs, hid):
    """-> wp [N_U, 128, KT*128] f16, bp [128, N_U] f32 for hidden shard
    `hid` (shared by the 4 batch-shard cores in that column).

    Unit seq order matches the device loop: (r, br, g in GATE_ORDER).
    Unit layout per partition row kk: [k(KT), m(128)] = 3072B, where
    wp[u][kk, k*128 + m] = W[u_row m, k*128 + kk].
    """
    rs = slice(hid * HS, (hid + 1) * HS)
    units = np.empty((RT, 2, 4, 128, KT, 128), dtype=F16)
    bps = np.empty((128, RT, 2, 4), dtype=np.float32)
    for bi, suffix in enumerate(("_light", "_light_temp")):
        for gi, g in enumerate(GATE_ORDER):
            gname = "fico"[g]
            Wg = inputs[f"w_{gname}{suffix}"][rs].astype(F16)   # [HS, K]
            bg = inputs[f"b_{gname}{suffix}"][rs]
            t = Wg.reshape(RT, 128, KT, 128)                    # [r, m, k, kk]
            units[:, bi, gi] = t.transpose(0, 3, 2, 1)          # [r, kk, k, m]
            bps[:, :, bi, gi] = bg.reshape(RT, 128).T
    wp = np.ascontiguousarray(units).reshape(N_U, 128, KT * 128)
    bp = np.ascontiguousarray(bps).reshape(128, N_U)
    return wp, bp


def _pack_core_inputs(inputs, wps, bps, core):
    b4, hid = divmod(core, SH)
    bsl = slice(b4 * BS, (b4 + 1) * BS)
    y = inputs["y"][bsl]
    out = {"wp": wps[hid], "bp": bps[hid]}
    a = np.empty((2, CH, 128, KT * CB), dtype=F16)
    for bi, hname in ((0, "h_light"), (1, "h_temp")):
        hx = np.concatenate([inputs[hname][bsl], y], axis=1).astype(F16)
        # [ch, p, k, j]: element = hx[ch*CB + j, k*128 + p]
        a2 = hx.reshape(CH, CB, KT, 128).transpose(0, 3, 2, 1)
        a[bi] = np.ascontiguousarray(a2).reshape(CH, 128, KT * CB)
    out["ap"] = a
    cl = inputs["c_light"][bsl, hid * HS:(hid + 1) * HS].astype(np.float32)
    # ct[r, ch, p, j] = c[ch*CB + j, r*128 + p]
    c4 = cl.reshape(CH, CB, RT, 128).transpose(2, 0, 3, 1)
    out["ct"] = np.ascontiguousarray(c4)
    return out


def make_in_maps(**inputs):
    wps, bps = zip(*[_pack_weights(inputs, hid) for hid in range(SH)])
    return [_pack_core_inputs(inputs, wps, bps, c) for c in range(N_CORES)]


def _unpack_core(res):
    # [r, ch, p, j] -> [ch*CB + j, r*128 + p]
    h = res["h_out"].transpose(1, 3, 0, 2).reshape(BS, HS)
    c = res["c_out"].transpose(1, 3, 0, 2).reshape(BS, HS)
    return h, c


def unpack_core0(res0):
    return _unpack_core(res0)


def unpack_results(results):
    h_new = np.empty((B, H), dtype=np.float32)
    c_new = np.empty((B, H), dtype=np.float32)
    for core, res in enumerate(results):
        b4, hid = divmod(core, SH)
        h, c = _unpack_core(res)
        h_new[b4 * BS:(b4 + 1) * BS, hid * HS:(hid + 1) * HS] = h
        c_new[b4 * BS:(b4 + 1) * BS, hid * HS:(hid + 1) * HS] = c
    return h_new, c_new


def kernel(**inputs):
    inputs = {k: np.asarray(v) for k, v in inputs.items()}
    nc = _get_nc()
    in_maps = make_in_maps(**inputs)
    res = run_bass_kernel_spmd(nc, in_maps, list(range(N_CORES)))
    return unpack_results(res.results)
